# revision 26
# baseline (speedup 1.0000x reference)
"""BERT-embedding kernel for Trainium2 (8 NeuronCores, data-parallel).

Computes, for input_sequence [256,512,10], doy_sequence [256,512] (int32),
W [256,10], b [256]:

    obs = input_sequence @ W.T + b          # [256,512,256]
    pos = PE_TABLE[doy_sequence]            # [256,512,256]
    out = concat([obs, pos], axis=-1)       # [256,512,512] fp32

Strategy: shard the batch dim 8 ways (32 batches / 16384 tokens per core).
The PE table is derived data (sinusoids of doy), so instead of gathering
rows from HBM (the old kernel burned ~138us of Q7 SWDGE descriptor time
plus 16MB/core of gather reads), each core COMPUTES the positional
embeddings on the fly:

  - One fp16 matmul per 128-token column produces, in PSUM,
    [obs(256) | sin-args(128) | cos-args(128)] per token: the lhsT
    carries [x features; 1; pos; pos; mask] and the rhs carries [W.T; b;
    div/2pi fp16-hi; div/2pi fp16-lo; cos offset 0.25] (args in turn
    units; pos<=365 is exact in fp16 and the hi/lo split keeps args
    fp32-grade). doy==0 rows get mask=0 so both halves hit sin(0)=0.
  - DVE range-reduces args to f in [-0.5,0.5] turns with the
    magic-number round trick (rr=(a+1.5*2^23)-1.5*2^23; f=a-rr) for the
    80/128 dim pairs that can wrap, and copies the 48 no-wrap pairs'
    args into the same full f tile.
  - ACT copies the obs half out of PSUM (at raised scheduler priority,
    so it overlaps DVE's reduction and lets the PSUM tile recycle
    early) and then evaluates ONE Sin(2pi*f) op over the full f tile,
    writing even/odd interleaved sin/cos straight into the combined
    output tile.
  - One HWDGE DMA per 512-token chunk writes the finished [128,4,512]
    tile, alternating between the sync and gpsimd rings; tokens are
    laid out so each SBUF partition holds 4 consecutive output rows
    (8KB contiguous in DRAM).

This leaves the kernel bound by the unavoidable 32MB/core fp32 output
write; per 512-token chunk the engines run ~2.1-2.4us under the ~2.6us
DMA service time.
"""

import math

import numpy as np

import concourse.bacc as bacc
import concourse.mybir as mybir
import concourse.tile as tile
from concourse.bass_utils import run_bass_kernel_spmd

F32 = mybir.dt.float32
F16 = mybir.dt.float16

# Problem shapes (hardcoded per the harness contract).
B, S, NF = 256, 512, 10
E = 256
ED2 = E // 2                      # 128 sin/cos dim pairs
MAX_LEN = 366
N_CORES = 8
TOK = (B // N_CORES) * S          # tokens per core = 16384
CPC = 4                           # 128-token cols per chunk
CH = CPC * 128                    # tokens per chunk = 512
NCH = TOK // CH                   # 32
KR = 14                           # lhsT rows: 10 features + ones + pos + pos + mask
DCUT = 80                         # dim pairs [0,DCUT) can wrap (need reduction)
TWO_PI = 2.0 * math.pi
MAGIC = 12582912.0                # 1.5 * 2**23: fp32 round-to-int trick

_COMPILED_NC = None
_LAST_RESULTS = None               # BassKernelResults of the most recent run


def _build():
    nc = bacc.Bacc("TRN2", target_bir_lowering=False, debug=False)
    # XL is packed [78, TOK/2]: even chunks' lhsT rows live at partitions
    # 0..13, odd chunks' at 64..77 (matmul base-partition constraint), so
    # the input load engages 78 partitions instead of 14.
    XL = nc.dram_tensor("XL", [64 + KR, TOK // 2], F16, kind="ExternalInput")
    RH = nc.dram_tensor("RH", [KR, 2 * E], F16, kind="ExternalInput")
    out = nc.dram_tensor("out", [TOK, 2 * E], F32, kind="ExternalOutput")

    # Token t = cc*512 + p*4 + j lives at partition p, col-group j of chunk
    # cc, so each partition's 4 col-groups are 4 consecutive DRAM rows.
    out5 = out.ap().rearrange("(cc p j) e -> cc p j e", p=128, j=CPC)

    with tile.TileContext(nc) as tc:
        with (
            tc.tile_pool(name="const", bufs=1) as cpool,
            tc.tile_pool(name="comb", bufs=6) as combpool,
            tc.tile_pool(name="red", bufs=3) as redpool,
            tc.tile_pool(name="psum", bufs=2, space="PSUM") as ppool,
        ):
            rh_sb = cpool.tile([64 + KR, 2 * E], F16, tag="rh_sb")
            nc.sync.dma_start(out=rh_sb[0:KR, :], in_=RH[:, :])
            nc.sync.dma_start(out=rh_sb[64 : 64 + KR, :], in_=RH[:, :])
            xl_sb = cpool.tile([64 + KR, TOK // 2], F16, tag="xl_sb")
            # Chunked load so early matmuls start before the full load lands;
            # each 512-col piece carries 2 chunks' worth of lhsT data.
            xl_cuts = [0, 512, 1536, 3584, 8192]
            for ci in range(len(xl_cuts) - 1):
                nc.scalar.dma_start(
                    out=xl_sb[:, xl_cuts[ci] : xl_cuts[ci + 1]],
                    in_=XL[:, xl_cuts[ci] : xl_cuts[ci + 1]],
                )

            for cc in range(NCH):
                ps = ppool.tile([128, CPC, 2 * E], F32, tag="ps")
                u, g = cc // 2, cc % 2
                for j in range(CPC):
                    c0 = u * 512 + j * 128
                    nc.tensor.matmul(
                        out=ps[:, j, :],
                        lhsT=xl_sb[64 * g : 64 * g + KR, c0 : c0 + 128],
                        rhs=rh_sb[64 * g : 64 * g + KR, :],
                        start=True,
                        stop=True,
                    )
                comb = combpool.tile([128, CPC, 2 * E], F32, tag="comb")
                # obs copy on ACT: it only needs the matmuls, so it can
                # overlap DVE's reduction and release the PSUM tile early.
                nc.scalar.copy(out=comb[:, :, 0:E], in_=ps[:, :, 0:E])

                # pos args viewed as (j, half, dim-pair)
                args = ps[:, :, E : 2 * E].rearrange("p j (h d) -> p j h d", h=2)
                rr = redpool.tile([128, CPC, 2, DCUT], F32, tag="rr")
                nc.vector.tensor_scalar(
                    out=rr[:],
                    in0=args[:, :, :, 0:DCUT],
                    scalar1=MAGIC,
                    scalar2=MAGIC,
                    op0=mybir.AluOpType.add,
                    op1=mybir.AluOpType.subtract,
                )
                f = redpool.tile([128, CPC, 2, ED2], F32, tag="f")
                nc.vector.tensor_tensor(
                    out=f[:, :, :, 0:DCUT],
                    in0=args[:, :, :, 0:DCUT],
                    in1=rr[:],
                    op=mybir.AluOpType.subtract,
                )
                nc.vector.tensor_copy(
                    out=f[:, :, :, DCUT:ED2], in_=args[:, :, :, DCUT:ED2]
                )
                # one Sin over the whole f tile, interleaving sin/cos on write
                nc.scalar.activation(
                    out=comb[:, :, E : 2 * E].rearrange(
                        "p j (d h) -> p j h d", d=ED2
                    ),
                    in_=f[:],
                    func=mybir.ActivationFunctionType.Sin,
                    scale=TWO_PI,
                )
                nc.sync.dma_start(out=out5[cc][0:64], in_=comb[0:64])
                nc.gpsimd.dma_start(out=out5[cc][64:128], in_=comb[64:128])
    nc.compile()
    return nc


def kernel(input_sequence, doy_sequence, W, b) -> np.ndarray:
    global _COMPILED_NC, _LAST_RESULTS

    x = np.asarray(input_sequence, dtype=np.float32)
    doy = np.asarray(doy_sequence, dtype=np.int32)
    W = np.asarray(W, dtype=np.float32)
    bias = np.asarray(b, dtype=np.float32)

    if _COMPILED_NC is None:
        _COMPILED_NC = _build()
    nc = _COMPILED_NC

    # Shared rhs [14, 512]: cols 0..255 obs = W.T rows + bias row; cols
    # 256..511 pos args: div/(2pi) split fp16 hi/lo, cos offset 0.25 turns.
    d2 = (
        np.exp(np.arange(0, E, 2, dtype=np.float32) * -(math.log(10000.0) / E))
        / TWO_PI
    ).astype(np.float32)
    d2h = d2.astype(np.float16)
    d2l = (d2 - d2h.astype(np.float32)).astype(np.float16)
    RHv = np.zeros((KR, 2 * E), np.float16)
    RHv[0:NF, 0:E] = W.T.astype(np.float16)
    RHv[NF, 0:E] = bias.astype(np.float16)
    RHv[NF + 1, E : E + ED2] = d2h
    RHv[NF + 1, E + ED2 :] = d2h
    RHv[NF + 2, E : E + ED2] = d2l
    RHv[NF + 2, E + ED2 :] = d2l
    RHv[NF + 3, E + ED2 :] = 0.25

    bpc = B // N_CORES
    in_maps = []
    for c in range(N_CORES):
        xc = x[c * bpc : (c + 1) * bpc].reshape(TOK, NF)
        dc = doy[c * bpc : (c + 1) * bpc].reshape(TOK)
        posf = np.where(dc == 0, 0, dc - 1).astype(np.float16)
        maskf = (dc != 0).astype(np.float16)
        XLv = np.empty((KR, TOK), np.float16)
        XLv[0:NF] = xc.T.astype(np.float16)
        XLv[NF] = 1.0
        XLv[NF + 1] = posf
        XLv[NF + 2] = posf
        XLv[NF + 3] = maskf
        # Device chunk cc=2u+g, col j, partition p holds token
        # t=cc*512+p*4+j; lhsT rows live at partitions 64*g..64*g+KR and
        # cols u*512+j*128+p of the packed [78, TOK/2] layout.
        XLv = XLv.reshape(KR, NCH, 128, CPC).transpose(0, 1, 3, 2)  # r,cc,j,p
        XLv = XLv.reshape(KR, NCH // 2, 2, CPC, 128).transpose(2, 0, 1, 3, 4)
        XLv = XLv.reshape(2 * KR, TOK // 2)
        XL78 = np.zeros((64 + KR, TOK // 2), np.float16)
        XL78[0:KR] = XLv[0:KR]
        XL78[64 : 64 + KR] = XLv[KR:]
        in_maps.append({"XL": XL78, "RH": RHv})

    _LAST_RESULTS = run_bass_kernel_spmd(nc, in_maps, core_ids=list(range(N_CORES)))

    out = np.empty((B, S, 2 * E), dtype=np.float32)
    for c in range(N_CORES):
        out[c * bpc : (c + 1) * bpc] = _LAST_RESULTS.results[c]["out"].reshape(
            bpc, S, 2 * E
        )
    return out


# revision 27
# speedup vs baseline: 1.1585x; 1.1585x over previous
"""BERT-embedding kernel for Trainium2 (8 NeuronCores, data-parallel).

Computes, for input_sequence [256,512,10], doy_sequence [256,512] (int32),
W [256,10], b [256]:

    obs = input_sequence @ W.T + b          # [256,512,256]
    pos = PE_TABLE[doy_sequence]            # [256,512,256]
    out = concat([obs, pos], axis=-1)       # [256,512,512] fp32

Strategy: shard the batch dim 8 ways (32 batches / 16384 tokens per core).
The PE table is derived data (sinusoids of doy), so instead of gathering
rows from HBM (the old kernel burned ~138us of Q7 SWDGE descriptor time
plus 16MB/core of gather reads), each core COMPUTES the positional
embeddings on the fly:

  - One fp16 matmul per 128-token column produces, in PSUM,
    [obs(256) | sin-args(128) | cos-args(128)] per token: the lhsT
    carries [x features; 1; pos; pos; mask] and the rhs carries [W.T; b;
    div/2pi fp16-hi; div/2pi fp16-lo; cos offset 0.25] (args in turn
    units; pos<=365 is exact in fp16 and the hi/lo split keeps args
    fp32-grade). doy==0 rows get mask=0 so both halves hit sin(0)=0.
  - DVE range-reduces args to f in [-0.5,0.5] turns with the
    magic-number round trick (rr=(a+1.5*2^23)-1.5*2^23; f=a-rr) for the
    80/128 dim pairs that can wrap, and copies the 48 no-wrap pairs'
    args into the same full f tile.
  - ACT copies the obs half out of PSUM (at raised scheduler priority,
    so it overlaps DVE's reduction and lets the PSUM tile recycle
    early) and then evaluates ONE Sin(2pi*f) op over the full f tile,
    writing even/odd interleaved sin/cos straight into the combined
    output tile.
  - One HWDGE DMA per 512-token chunk writes the finished [128,4,512]
    tile, alternating between the sync and gpsimd rings; tokens are
    laid out so each SBUF partition holds 4 consecutive output rows
    (8KB contiguous in DRAM).

This leaves the kernel bound by the unavoidable 32MB/core fp32 output
write; per 512-token chunk the engines run ~2.1-2.4us under the ~2.6us
DMA service time.
"""

import math

import numpy as np

import concourse.bacc as bacc
import concourse.mybir as mybir
import concourse.tile as tile
from concourse.bass_utils import run_bass_kernel_spmd

F32 = mybir.dt.float32
F16 = mybir.dt.float16

# Problem shapes (hardcoded per the harness contract).
B, S, NF = 256, 512, 10
E = 256
ED2 = E // 2                      # 128 sin/cos dim pairs
MAX_LEN = 366
N_CORES = 8
TOK = (B // N_CORES) * S          # tokens per core = 16384
CPC = 4                           # 128-token cols per chunk
CH = CPC * 128                    # tokens per chunk = 512
NCH = TOK // CH                   # 32
KR = 14                           # lhsT rows: 10 features + ones + pos + pos + mask
DCUT = 80                         # dim pairs [0,DCUT) can wrap (need reduction)
TWO_PI = 2.0 * math.pi
MAGIC = 12582912.0                # 1.5 * 2**23: fp32 round-to-int trick

_COMPILED_NC = None
_LAST_RESULTS = None               # BassKernelResults of the most recent run


def _build():
    nc = bacc.Bacc("TRN2", target_bir_lowering=False, debug=False)
    # XL is packed [78, TOK/2]: even chunks' lhsT rows live at partitions
    # 0..13, odd chunks' at 64..77 (matmul base-partition constraint), so
    # the input load engages 78 partitions instead of 14.
    XL = nc.dram_tensor("XL", [64 + KR, TOK // 2], F16, kind="ExternalInput")
    RH = nc.dram_tensor("RH", [KR, 2 * E], F16, kind="ExternalInput")
    out = nc.dram_tensor("out", [TOK, 2 * E], F32, kind="ExternalOutput")

    # Token t = cc*512 + p*4 + j lives at partition p, col-group j of chunk
    # cc, so each partition's 4 col-groups are 4 consecutive DRAM rows.
    out5 = out.ap().rearrange("(cc p j) e -> cc p j e", p=128, j=CPC)

    with tile.TileContext(nc) as tc:
        with (
            tc.tile_pool(name="const", bufs=1) as cpool,
            tc.tile_pool(name="comb", bufs=6) as combpool,
            tc.tile_pool(name="red", bufs=3) as redpool,
            tc.tile_pool(name="psum", bufs=2, space="PSUM") as ppool,
        ):
            rh_sb = cpool.tile([64 + KR, 2 * E], F16, tag="rh_sb")
            nc.sync.dma_start(out=rh_sb[0:KR, :], in_=RH[:, :])
            nc.sync.dma_start(out=rh_sb[64 : 64 + KR, :], in_=RH[:, :])
            xl_sb = cpool.tile([64 + KR, TOK // 2], F16, tag="xl_sb")
            # Chunked load so early matmuls start before the full load lands;
            # each 512-col piece carries 2 chunks' worth of lhsT data.
            xl_cuts = [0, 512, 1536, 3584, 8192]
            for ci in range(len(xl_cuts) - 1):
                nc.scalar.dma_start(
                    out=xl_sb[:, xl_cuts[ci] : xl_cuts[ci + 1]],
                    in_=XL[:, xl_cuts[ci] : xl_cuts[ci + 1]],
                )

            for cc in range(NCH):
                ps = ppool.tile([128, CPC, 2 * E], F32, tag="ps")
                u, g = cc // 2, cc % 2
                for j in range(CPC):
                    c0 = u * 512 + j * 128
                    nc.tensor.matmul(
                        out=ps[:, j, :],
                        lhsT=xl_sb[64 * g : 64 * g + KR, c0 : c0 + 128],
                        rhs=rh_sb[64 * g : 64 * g + KR, :],
                        start=True,
                        stop=True,
                    )
                comb = combpool.tile([128, CPC, 2 * E], F32, tag="comb")
                # obs copy on ACT: it only needs the matmuls, so it can
                # overlap DVE's reduction and release the PSUM tile early.
                nc.scalar.copy(out=comb[:, :, 0:E], in_=ps[:, :, 0:E])

                # pos args viewed as (j, half, dim-pair)
                args = ps[:, :, E : 2 * E].rearrange("p j (h d) -> p j h d", h=2)
                rr = redpool.tile([128, CPC, 2, DCUT], F32, tag="rr")
                nc.vector.tensor_scalar(
                    out=rr[:],
                    in0=args[:, :, :, 0:DCUT],
                    scalar1=MAGIC,
                    scalar2=MAGIC,
                    op0=mybir.AluOpType.add,
                    op1=mybir.AluOpType.subtract,
                )
                f = redpool.tile([128, CPC, 2, ED2], F32, tag="f")
                nc.vector.tensor_tensor(
                    out=f[:, :, :, 0:DCUT],
                    in0=args[:, :, :, 0:DCUT],
                    in1=rr[:],
                    op=mybir.AluOpType.subtract,
                )
                nc.vector.tensor_copy(
                    out=f[:, :, :, DCUT:ED2], in_=args[:, :, :, DCUT:ED2]
                )
                # one Sin over the whole f tile, interleaving sin/cos on write
                nc.scalar.activation(
                    out=comb[:, :, E : 2 * E].rearrange(
                        "p j (d h) -> p j h d", d=ED2
                    ),
                    in_=f[:],
                    func=mybir.ActivationFunctionType.Sin,
                    scale=TWO_PI,
                )
                nc.sync.dma_start(out=out5[cc], in_=comb[:])
    nc.compile()
    return nc


def kernel(input_sequence, doy_sequence, W, b) -> np.ndarray:
    global _COMPILED_NC, _LAST_RESULTS

    x = np.asarray(input_sequence, dtype=np.float32)
    doy = np.asarray(doy_sequence, dtype=np.int32)
    W = np.asarray(W, dtype=np.float32)
    bias = np.asarray(b, dtype=np.float32)

    if _COMPILED_NC is None:
        _COMPILED_NC = _build()
    nc = _COMPILED_NC

    # Shared rhs [14, 512]: cols 0..255 obs = W.T rows + bias row; cols
    # 256..511 pos args: div/(2pi) split fp16 hi/lo, cos offset 0.25 turns.
    d2 = (
        np.exp(np.arange(0, E, 2, dtype=np.float32) * -(math.log(10000.0) / E))
        / TWO_PI
    ).astype(np.float32)
    d2h = d2.astype(np.float16)
    d2l = (d2 - d2h.astype(np.float32)).astype(np.float16)
    RHv = np.zeros((KR, 2 * E), np.float16)
    RHv[0:NF, 0:E] = W.T.astype(np.float16)
    RHv[NF, 0:E] = bias.astype(np.float16)
    RHv[NF + 1, E : E + ED2] = d2h
    RHv[NF + 1, E + ED2 :] = d2h
    RHv[NF + 2, E : E + ED2] = d2l
    RHv[NF + 2, E + ED2 :] = d2l
    RHv[NF + 3, E + ED2 :] = 0.25

    bpc = B // N_CORES
    in_maps = []
    for c in range(N_CORES):
        xc = x[c * bpc : (c + 1) * bpc].reshape(TOK, NF)
        dc = doy[c * bpc : (c + 1) * bpc].reshape(TOK)
        posf = np.where(dc == 0, 0, dc - 1).astype(np.float16)
        maskf = (dc != 0).astype(np.float16)
        XLv = np.empty((KR, TOK), np.float16)
        XLv[0:NF] = xc.T.astype(np.float16)
        XLv[NF] = 1.0
        XLv[NF + 1] = posf
        XLv[NF + 2] = posf
        XLv[NF + 3] = maskf
        # Device chunk cc=2u+g, col j, partition p holds token
        # t=cc*512+p*4+j; lhsT rows live at partitions 64*g..64*g+KR and
        # cols u*512+j*128+p of the packed [78, TOK/2] layout.
        XLv = XLv.reshape(KR, NCH, 128, CPC).transpose(0, 1, 3, 2)  # r,cc,j,p
        XLv = XLv.reshape(KR, NCH // 2, 2, CPC, 128).transpose(2, 0, 1, 3, 4)
        XLv = XLv.reshape(2 * KR, TOK // 2)
        XL78 = np.zeros((64 + KR, TOK // 2), np.float16)
        XL78[0:KR] = XLv[0:KR]
        XL78[64 : 64 + KR] = XLv[KR:]
        in_maps.append({"XL": XL78, "RH": RHv})

    _LAST_RESULTS = run_bass_kernel_spmd(nc, in_maps, core_ids=list(range(N_CORES)))

    out = np.empty((B, S, 2 * E), dtype=np.float32)
    for c in range(N_CORES):
        out[c * bpc : (c + 1) * bpc] = _LAST_RESULTS.results[c]["out"].reshape(
            bpc, S, 2 * E
        )
    return out
